# revision 23
# baseline (speedup 1.0000x reference)
"""Trainium2 Bass kernel for the dual-branch CustomLSTMCell — Strassen edition.

Sharding: 2-D over 8 cores — batch 4096 -> 4 quarters (NB=1024) x hidden
1024 -> 2 halves (NH=512).  Each core computes h_new/c_new for its
[1024 batch x 512 h] block; no collectives.  Per core per branch the GEMM is
    C[2048 zc, 1024 b] = Wb[2048, 1536] @ hx_br.T[1536, 1024]
with zc rows = [f(512) i(512) | c(512) o(512)] gates' h-slices.

One level of Strassen (M=1024, K=768, N=512 splits) turns 8 block-multiplies
into 7 — PE matmul cycles drop 8->7 (768 -> 672 512-wide matmuls/core):
    M1=(A11+A22)(B11+B22) M2=(A21+A22)B11 M3=A11(B12-B22) M4=A22(B21-B11)
    M5=(A11+A12)B22 M6=(A21-A11)(B11+B12) M7=(A12-A22)(B21+B22)
    C11=M1+M4-M5+M7  C12=M3+M5  C21=M2+M4  C22=M1-M2+M3+M6
A-side combos are packed on the host (fp16, streamed once: 22MB/core vs
25.2MB for the replicated-weight classic kernel).  B-side combos are built
on-device by the Vector engine from the plain activations (so the DMA only
carries the 5.2MB of plain hx, with the y-part shared between branches).
M1..M7 for one 128-row i-tile live in 7 PSUM banks; the Vector engine
recombines them into C (3 ScalarE psum->sbuf copies + 8 fused
scalar_tensor_tensor ops), ScalarE applies bias+sigmoid/tanh straight to
bf16 gate tiles, GpSimd does the LSTM cell elementwise, outputs stream back
as bf16 [h, batch] tiles that the host transposes/upcasts.

Branch order: all light-branch i-tiles first, then temp-branch (so a_t has
~80us to arrive); i-tiles pair (s, s+4) so each h-slice s gets f,i,c,o of
both branches, and cell(s) fires during the temp branch as pairs complete.
"""

import os
import sys

for _p in ("/opt/trn_rl_repo",):
    if os.path.isdir(_p) and _p not in sys.path:
        sys.path.append(_p)

import numpy as np
import ml_dtypes

import concourse.bass as bass
import concourse.mybir as mybir
import concourse.tile as tile
from concourse import bacc
from concourse.bass_utils import run_bass_kernel_spmd

B, I, H = 4096, 512, 1024
N_CORES = 8
NB = 1024                  # batch rows per core (4 quarters)
NH = 512                   # h cols per core (2 halves)
K = H + I                  # 1536
KT = 12
KH = 6                     # k-tiles per Strassen half
N_ROWS = 106               # rows: 2 br x (7 Strassen tiles x 7 + 1 classic x 4)
J_ORDER = (2, 5, 1, 3, 4, 6, 7)   # plain-B multiplies first
PF = 9                     # A-row prefetch depth
N_WARM = 5

_F32 = mybir.dt.float32
_F16 = mybir.dt.float16
_BF16 = mybir.dt.bfloat16
AF = mybir.ActivationFunctionType
ALU = mybir.AluOpType
F16 = np.float16
BF16 = ml_dtypes.bfloat16


def _build_nc():
    nc = bacc.Bacc("TRN2", target_bir_lowering=False, debug=False,
                   enable_asserts=False)

    wpa = nc.dram_tensor("wpa", [N_ROWS, 128, KH * 128], _F16,
                         kind="ExternalInput")
    a_l = nc.dram_tensor("a_l", [KT, 128, NB], _F16, kind="ExternalInput")
    a_t = nc.dram_tensor("a_t", [8, 128, NB], _F16, kind="ExternalInput")
    bp = nc.dram_tensor("bp", [128, 32], _F32, kind="ExternalInput")
    ct = nc.dram_tensor("ct", [4, 128, NB], _BF16, kind="ExternalInput")
    h_out = nc.dram_tensor("h_out", [4, 128, NB], _BF16, kind="ExternalOutput")
    c_out = nc.dram_tensor("c_out", [4, 128, NB], _BF16, kind="ExternalOutput")

    with tile.TileContext(nc) as tc:
        with (
            tc.tile_pool(name="const", bufs=1) as cpool,
            tc.tile_pool(name="w", bufs=PF + 1) as wpool,
            tc.tile_pool(name="bc", bufs=5) as bcpool,
            tc.tile_pool(name="cg", bufs=2) as cgpool,
            tc.tile_pool(name="ms", bufs=2) as mspool,
            tc.tile_pool(name="gates", bufs=20) as gpool,
            tc.tile_pool(name="cin", bufs=4) as cin_pool,
            tc.tile_pool(name="ew", bufs=2) as epool,
            tc.tile_pool(name="out", bufs=2) as opool,
            tc.tile_pool(name="psum", bufs=8, space="PSUM") as pspool,
        ):
            # ---- PE pre-warm ------------------------------------------------
            warm = cpool.tile([128, 512], _F16, tag="warm")
            nc.vector.memset(warm[:], 0.0)
            def warm_fill(n):
                wt_ = pspool.tile([128, 512], _F32, tag="mp")
                for _ in range(n):
                    nc.tensor.matmul(wt_[:], warm[:, 0:128], warm[:],
                                     start=True, stop=True)

            warm_fill(N_WARM)

            # ---- input DMAs -------------------------------------------------
            # sync: a_l sequential (first i-tile chases it); then bias, ct.
            # scalar: even A-rows.  gpsimd: a_t then odd A-rows.
            asb_l = cpool.tile([128, KT * NB], _F16, tag="asb_l")
            asb_t = cpool.tile([128, 8 * NB], _F16, tag="asb_t")
            for bh in (0, 1):
                for kt in range(KT):
                    eng = nc.sync if kt % 2 == 0 else nc.gpsimd
                    b0 = bh * 512
                    eng.dma_start(out=asb_l[:, kt * NB + b0: kt * NB + b0 + 512],
                                  in_=a_l[kt][:, b0:b0 + 512])
            bias_sb = cpool.tile([128, 32], _F32, tag="bias")
            nc.sync.dma_start(out=bias_sb[:], in_=bp[:])
            ct_sb = [None] * 4

            def issue_ct():
                for s in range(4):
                    t = cin_pool.tile([128, NB], _BF16, tag="ct")
                    nc.gpsimd.dma_start(out=t[:], in_=ct[s])
                    ct_sb[s] = t

            def issue_at():
                for kt in range(8):
                    nc.gpsimd.dma_start(out=asb_t[:, bass.ts(kt, NB)],
                                        in_=a_t[kt])

            # ---- A-row stream ----------------------------------------------
            wrows = {}

            _ENGS = (nc.scalar, nc.gpsimd, nc.sync)

            def issue_row(n):
                t = wpool.tile([128, KH * 128], _F16, tag="wr")
                eng = nc.scalar if n < 7 else _ENGS[n % 3]
                eng.dma_start(out=t[:], in_=wpa[n])
                wrows[n] = t

            for n in range(PF):
                issue_row(n)

            # ---- helpers ----------------------------------------------------
            def a_slice(br, kt, b0, b1):
                """[128, b1-b0] slice of k-tile kt of branch br's acts."""
                if br == 0 or kt >= 8:
                    return asb_l[:, kt * NB + b0: kt * NB + b1]
                return asb_t[:, kt * NB + b0: kt * NB + b1]

            bc_tiles = {}

            def build_bc(br, kt):
                """Vector-engine B-combos for one k-tile (fp16)."""
                for j in (1, 3, 4, 6, 7):
                    if (br, j) not in bc_tiles:
                        t = bcpool.tile([128, KH * 512], _F16, tag="bc")
                        bc_tiles[(br, j)] = t
                lo1 = a_slice(br, kt, 0, 512)
                lo2 = a_slice(br, kt, 512, 1024)
                hi1 = a_slice(br, kt + 6, 0, 512)
                hi2 = a_slice(br, kt + 6, 512, 1024)
                d = {k: bc_tiles[(br, k)][:, bass.ts(kt, 512)] for k in (1, 3, 4, 6, 7)}
                nc.vector.tensor_add(d[1], lo1, hi2)   # B11+B22
                nc.vector.tensor_sub(d[3], lo2, hi2)   # B12-B22
                nc.vector.tensor_sub(d[4], hi1, lo1)   # B21-B11
                nc.vector.tensor_add(d[6], lo1, lo2)   # B11+B12
                nc.vector.tensor_add(d[7], hi1, hi2)   # B21+B22
            for kt in range(KH):
                build_bc(0, kt)

            def rhs(br, j, kt):
                if j == 2:
                    return a_slice(br, kt, 0, 512)
                if j == 5:
                    return a_slice(br, kt + 6, 512, 1024)
                return bc_tiles[(br, j)][:, bass.ts(kt, 512)]

            gates = {}
            row_n = [0]

            def do_itile_classic(br, s, ii, tc_idx):
                # plain 2-half GEMM for this row-tile pair: no B-combos (so it
                # can chase the act stream k-tile by k-tile at startup) and no
                # recombine (gates activate straight from PSUM at the tail).
                rows = []
                for _ in range(4):
                    n = row_n[0]
                    row_n[0] += 1
                    if n + PF < N_ROWS:
                        issue_row(n + PF)
                    rows.append(wrows.pop(n))
                g1 = 'f' if ii == s else 'i'
                g2 = 'c' if ii == s else 'o'
                gt1 = gpool.tile([128, NB], _BF16, tag="g", bufs=20)
                gt2 = gpool.tile([128, NB], _BF16, tag="g", bufs=20)
                gt = {1: gt1, 2: gt2}
                border = (0, 512) if br == 0 else (512, 0)
                for b0 in border:
                    for half, gname in ((1, g1), (2, g2)):
                        pt = pspool.tile([128, 512], _F32, tag="mp")
                        for kt in range(KT):
                            wt = rows[(half - 1) * 2 + kt // KH]
                            nc.tensor.matmul(
                                pt[:], wt[:, bass.ts(kt % KH, 128)],
                                a_slice(br, kt, b0, b0 + 512),
                                start=(kt == 0), stop=(kt == KT - 1))
                        func = AF.Tanh if gname == 'c' else AF.Sigmoid
                        col = tc_idx * 2 + (half - 1)
                        nc.scalar.activation(gt[half][:, b0:b0 + 512], pt[:],
                                             func,
                                             bias=bias_sb[:, col:col + 1],
                                             scale=1.0)

                gates[(br, g1, s)] = gt[1]
                gates[(br, g2, s)] = gt[2]

            def do_itile(br, s, ii, tc_idx):
                mp = {}
                for j in J_ORDER:
                    n = row_n[0]
                    row_n[0] += 1
                    if n + PF < N_ROWS:
                        issue_row(n + PF)
                    wt = wrows.pop(n)
                    pt = pspool.tile([128, 512], _F32, tag="mp")
                    for kt in range(KH):
                        nc.tensor.matmul(pt[:], wt[:, bass.ts(kt, 128)],
                                         rhs(br, j, kt),
                                         start=(kt == 0), stop=(kt == KH - 1))
                    mp[j] = pt

                # recombine 7 M's -> C (2x [128, 1024] f32)
                m1 = mspool.tile([128, 512], _F32, tag="m1")
                m3 = mspool.tile([128, 512], _F32, tag="m3")
                m4 = mspool.tile([128, 512], _F32, tag="m4")
                nc.scalar.copy(m1[:], mp[1][:])
                nc.scalar.copy(m3[:], mp[3][:])
                nc.scalar.copy(m4[:], mp[4][:])
                cg1 = cgpool.tile([128, 1024], _F32, tag="cg1")
                cg2 = cgpool.tile([128, 1024], _F32, tag="cg2")
                t = mspool.tile([128, 512], _F32, tag="t", bufs=1)
                y = mspool.tile([128, 512], _F32, tag="y", bufs=1)
                V = nc.vector
                V.scalar_tensor_tensor(t[:], mp[5][:], -1.0, m1[:],
                                       ALU.mult, ALU.add)          # m1-M5
                V.tensor_add(t[:], t[:], mp[4][:])                 # +M4
                V.scalar_tensor_tensor(cg1[:, 0:512], mp[7][:], 0.0, t[:],
                                       ALU.add, ALU.add)           # C11
                V.scalar_tensor_tensor(cg1[:, 512:1024], mp[5][:], 0.0, m3[:],
                                       ALU.add, ALU.add)           # C12
                V.scalar_tensor_tensor(cg2[:, 0:512], mp[2][:], 0.0, m4[:],
                                       ALU.add, ALU.add)           # C21
                V.scalar_tensor_tensor(y[:], mp[2][:], -1.0, m1[:],
                                       ALU.mult, ALU.add)          # m1-M2
                V.tensor_add(y[:], y[:], m3[:])                    # +M3
                V.scalar_tensor_tensor(cg2[:, 512:1024], mp[6][:], 0.0, y[:],
                                       ALU.add, ALU.add)           # C22

                # gate activations (bias per-partition), bf16 outputs
                g1 = 'f' if ii == s else 'i'
                g2 = 'c' if ii == s else 'o'
                for half, (cg, gname) in enumerate(((cg1, g1), (cg2, g2))):
                    gt = gpool.tile([128, NB], _BF16, tag="g")
                    func = AF.Tanh if gname == 'c' else AF.Sigmoid
                    col = tc_idx * 2 + half
                    nc.scalar.activation(gt[:], cg[:], func,
                                         bias=bias_sb[:, col:col + 1],
                                         scale=1.0)
                    gates[(br, gname, s)] = gt

            def cell_partial(s):
                # after br1 (s+4): i2/o2 known; precompute i1*ch1 and o1+o2
                i1 = gates[(0, 'i', s)]; ch1 = gates[(0, 'c', s)]
                o1 = gates[(0, 'o', s)]; o2 = gates[(1, 'o', s)]
                G = nc.vector
                pp = epool.tile([128, NB], _BF16, tag="pre", bufs=2)
                u = epool.tile([128, NB], _BF16, tag="uu", bufs=2)
                G.tensor_mul(pp[:], i1[:], ch1[:])
                G.tensor_add(u[:], o1[:], o2[:])
                return pp, u

            def cell_final(s, pu, last=False):
                # after br1 (s): f2/ch2 known; finish c_new and h_new
                pp, u = pu
                f1 = gates[(0, 'f', s)]; f2 = gates[(1, 'f', s)]
                i2 = gates[(1, 'i', s)]; ch2 = gates[(1, 'c', s)]
                chunks = ((512, NB), (0, 512)) if last else ((0, NB),)
                for ci, (b0, b1) in enumerate(chunks):
                    w = b1 - b0
                    G = nc.vector
                    t2 = epool.tile([128, w], _F32, tag="ca", bufs=2)
                    v = epool.tile([128, w], _F32, tag="cb", bufs=2)
                    cn = opool.tile([128, w], _BF16, tag="cn")
                    th = epool.tile([128, w], _BF16, tag="th")
                    hn = opool.tile([128, w], _BF16, tag="hn")
                    G.tensor_mul(t2[:], i2[:, b0:b1], ch2[:, b0:b1])
                    G.tensor_add(v[:], f1[:, b0:b1], f2[:, b0:b1])
                    G.tensor_mul(v[:], v[:], ct_sb[s][:, b0:b1])
                    G.tensor_add(t2[:], t2[:], pp[:, b0:b1])
                    G.tensor_add(cn[:], t2[:], v[:])
                    nc.scalar.activation(th[:], cn[:], AF.Tanh)
                    G.tensor_mul(hn[:], u[:, b0:b1], th[:])
                    ceng = nc.sync if not last else (nc.sync if ci == 0 else nc.gpsimd)
                    heng = nc.sync if not last else (nc.sync if ci == 0 else nc.scalar)
                    ceng.dma_start(out=c_out[s][:, b0:b1], in_=cn[:])
                    heng.dma_start(out=h_out[s][:, b0:b1], in_=hn[:])

            # ---- main loop --------------------------------------------------
            tc_idx = 0
            pres = {}
            for br in range(2):
                if br == 1:
                    # temp-branch B-combos; dummy matmuls keep the PE (and
                    # the HAM clock-gate) busy while the Vector engine builds
                    for kt in range(KH):
                        build_bc(1, kt)
                    bwarm = pspool.tile([128, 512], _F32, tag="mp")
                    for _ in range(10):
                        nc.tensor.matmul(bwarm[:], warm[:, 0:128], warm[:],
                                         start=True, stop=True)
                for s in range(4):
                    pair = (s, s + 4) if br == 0 else (s + 4, s)
                    for ii in pair:
                        classic = (br == 0 and s == 0 and ii == 0) or \
                                  (br == 1 and s == 3 and ii == 3)
                        if classic:
                            do_itile_classic(br, s, ii, tc_idx)
                        else:
                            do_itile(br, s, ii, tc_idx)
                        tc_idx += 1
                        if br == 1:
                            if ii == s + 4:
                                pres[s] = cell_partial(s)
                            else:
                                cell_final(s, pres.pop(s), last=(s == 3))
                    if br == 0 and s == 1:
                        issue_at()
                    if br == 0 and s == 2:
                        issue_ct()

    nc.compile()
    return nc


_NC_CACHE = None


def _get_nc():
    global _NC_CACHE
    if _NC_CACHE is None:
        _NC_CACHE = _build_nc()
    return _NC_CACHE


def _pack_weights(inputs, hi):
    """A-combo rows + bias for h-half hi (shared by the 4 batch cores)."""
    rows = np.empty((N_ROWS, 128, KH * 128), dtype=F16)
    bias = np.empty((128, 32), dtype=np.float32)
    hsl = slice(hi * NH, (hi + 1) * NH)
    n = 0
    tc_idx = 0
    for br, suffix in enumerate(("_light", "_light_temp")):
        Wb = np.concatenate([inputs["w_" + g + suffix][hsl]
                             for g in "fico"], axis=0)      # [2048, 1536]
        bb = {g: inputs["b_" + g + suffix][hsl] for g in "fico"}
        A11, A12 = Wb[:1024, :768], Wb[:1024, 768:]
        A21, A22 = Wb[1024:, :768], Wb[1024:, 768:]
        P = {1: A11 + A22, 2: A21 + A22, 3: A11, 4: A22,
             5: A11 + A12, 6: A21 - A11, 7: A12 - A22}
        Pt = {j: np.ascontiguousarray(
                  p.reshape(8, 128, KH, 128).transpose(0, 3, 2, 1)
              ).astype(F16) for j, p in P.items()}          # [ii, kk, kt, m]
        Wt = np.ascontiguousarray(
            Wb.reshape(16, 128, KT, 128).transpose(0, 3, 2, 1)).astype(F16)
        for s in range(4):
            pair = (s, s + 4) if br == 0 else (s + 4, s)
            for ii in pair:
                classic = (br == 0 and s == 0 and ii == 0) or \
                          (br == 1 and s == 3 and ii == 3)
                if classic:
                    for half in (0, 8):
                        for kh in (0, 1):
                            rows[n] = Wt[half + ii][:, kh * KH:(kh + 1) * KH
                                                    ].reshape(128, KH * 128)
                            n += 1
                else:
                    for j in J_ORDER:
                        rows[n] = Pt[j][ii].reshape(128, KH * 128)
                        n += 1
                g1 = 'f' if ii == s else 'i'
                g2 = 'c' if ii == s else 'o'
                bias[:, tc_idx * 2] = bb[g1][s * 128:(s + 1) * 128]
                bias[:, tc_idx * 2 + 1] = bb[g2][s * 128:(s + 1) * 128]
                tc_idx += 1
    assert n == N_ROWS
    return rows, bias


def _pack_core_inputs(inputs, wps, bps, core):
    bi, hi = core // 2, core % 2
    bsl = slice(bi * NB, (bi + 1) * NB)
    y = inputs["y"][bsl]
    out = {"wpa": wps[hi], "bp": bps[hi]}
    hx = np.concatenate([inputs["h_light"][bsl], y], axis=1).astype(F16)
    out["a_l"] = np.ascontiguousarray(
        hx.reshape(NB, KT, 128).transpose(1, 2, 0))
    out["a_t"] = np.ascontiguousarray(
        inputs["h_temp"][bsl].astype(F16).reshape(NB, 8, 128).transpose(1, 2, 0))
    cl = inputs["c_light"][bsl, hi * NH:(hi + 1) * NH]      # [NB, NH]
    out["ct"] = np.ascontiguousarray(
        cl.reshape(NB, 4, 128).transpose(1, 2, 0)).astype(BF16)
    return out


def make_in_maps(**inputs):
    wps = [None, None]
    bps = [None, None]
    for hi in range(2):
        wps[hi], bps[hi] = _pack_weights(inputs, hi)
    return [_pack_core_inputs(inputs, wps, bps, c) for c in range(N_CORES)]


def _unpack_core(res):
    h = np.asarray(res["h_out"], dtype=np.float32)          # [4, 128, NB]
    c = np.asarray(res["c_out"], dtype=np.float32)
    h = h.transpose(2, 0, 1).reshape(NB, NH)
    c = c.transpose(2, 0, 1).reshape(NB, NH)
    return h, c


def unpack_core0(res0):
    return _unpack_core(res0)


def unpack_results(results):
    h_new = np.empty((B, H), dtype=np.float32)
    c_new = np.empty((B, H), dtype=np.float32)
    for core, res in enumerate(results):
        bi, hi = core // 2, core % 2
        h, c = _unpack_core(res)
        h_new[bi * NB:(bi + 1) * NB, hi * NH:(hi + 1) * NH] = h
        c_new[bi * NB:(bi + 1) * NB, hi * NH:(hi + 1) * NH] = c
    return h_new, c_new


def kernel(**inputs):
    inputs = {k: np.asarray(v) for k, v in inputs.items()}
    nc = _get_nc()
    in_maps = make_in_maps(**inputs)
    res = run_bass_kernel_spmd(nc, in_maps, list(range(N_CORES)))
    return unpack_results(res.results)
